# revision 9
# baseline (speedup 1.0000x reference)
"""Trainium2 Bass kernel for 3x3 same-padded conv (NCHW) scaled by 1/9.

v6: 1D Winograd F(4,3) along W (points {0, 2, -2, 1/2, -1/2}), bf16,
host-side input transform, de-interleaved contiguous output layout.

  - Data-parallel over batch: 8 NeuronCores x 4 images each (SPMD).
  - F(4,3): 6 products per 4 outputs -> 2x less PE work than direct conv
    (per-core PE floor 94 us vs 188 us direct bf16).
      f(x) = x^5 - 4.25 x^3 + x
      V_i = BT d (HOST, fp32 -> bf16), U_i = G w (1/9 folded, HOST)
      M_i(y,t) = sum_dy sum_ic U_i(dy) V_i(y+dy, t)      <- PE, PSUM fp32
      Y_a(y,t) = sum_i AT[a,i] M_i(y,t) = out(y, 4t+a)   <- DVE/Act/Pool
  - Points {0,2,-2,.5,-.5} chosen by CPU sweep: rel err 8.3e-3 (vs 1.4e-2
    for classic {0,1,-1,2,-2}); gate 2e-2.
  - Output written contiguously as [oc, y, a, t] fp32; host de-interleaves
    to [oc, y, 4t+a] (pure layout change, no arithmetic).
  - Transform engine split (v4b-proven): ScalarE 7 ops (copies + scales),
    DVE 7 ops (all PSUM readers + stt), GpSimd 3 SBUF adds. Constraints:
    tensor_tensor max 1 PSUM input; GpSimd no PSUM; stt only on DVE.
  - Loop order (img, chunk, oct): both oct groups of a chunk reuse the same
    V rows, so the PE never waits on late rows during startup.
  - DMA: u in (oct, i) pieces; img0's V in 6 fine row-pieces striped across
    both queues; imgs 1-3 in halves cut at row 31 (chunk0 needs rows 0..29).
    Output DMAs alternate queues; the last image splits its second chunk in
    two 14-row chunks to shorten the drain tail.
"""

import numpy as np
import ml_dtypes

import concourse.bacc as bacc
import concourse.mybir as mybir
import concourse.tile as tile
from concourse.bass_utils import run_bass_kernel_spmd

N_CORES = 8
N, IC, H, W = 32, 256, 56, 56
OC, KH, KW = 256, 3, 3
NPC = N // N_CORES
ICT = IC // 128
OCT = OC // 128
HP = H + 2
TQ = 14
NI = 6
NA = 4

BF16 = mybir.dt.bfloat16
F32 = mybir.dt.float32
MUL = mybir.AluOpType.mult
ADD = mybir.AluOpType.add

BT = np.array([
    [1, 0, -4.25, 0, 1, 0],
    [0, -0.5, -0.25, 2, 1, 0],
    [0, 0.5, -0.25, -2, 1, 0],
    [0, -2, -4, 0.5, 1, 0],
    [0, 2, -4, -0.5, 1, 0],
    [0, 1, 0, -4.25, 0, 1],
], np.float32)
G = np.array([
    [1, 0, 0],
    [1 / 30, 2 / 30, 4 / 30],
    [1 / 30, -2 / 30, 4 / 30],
    [-8 / 15, -4 / 15, -2 / 15],
    [-8 / 15, 4 / 15, -2 / 15],
    [0, 0, 1],
], np.float32)

_compiled = None


def _build():
    nc = bacc.Bacc("TRN2", target_bir_lowering=False, debug=False,
                   num_devices=N_CORES)

    v_d = nc.dram_tensor("v", [NPC, 128, ICT, NI, HP, TQ], BF16,
                         kind="ExternalInput")
    u_d = nc.dram_tensor("u", [128, OCT, NI, KH, ICT, 128], BF16,
                         kind="ExternalInput")
    o_d = nc.dram_tensor("out", [NPC, OC, H, NA, TQ], F32,
                         kind="ExternalOutput")

    with tile.TileContext(nc) as tc:
        with (
            tc.tile_pool(name="vp", bufs=1) as vpool,
            tc.tile_pool(name="up", bufs=1) as upool,
            tc.tile_pool(name="tp", bufs=4) as tpool,
            tc.tile_pool(name="op", bufs=4) as opool,
            tc.tile_pool(name="ps", bufs=8, space="PSUM") as pspool,
        ):
            usb = upool.tile([128, OCT, NI, KH, ICT, 128], BF16, name="usb")
            vt = []
            for img in range(NPC):
                vt.append(vpool.tile([128, ICT, NI, HP, TQ], BF16,
                                     tag=f"v{img}", name=f"v{img}"))

            for i in range(0, NI, 2):
                nc.sync.dma_start(usb[:, 0, i], u_d[:, 0, i])
                nc.scalar.dma_start(usb[:, 0, i + 1], u_d[:, 0, i + 1])
            cuts = [0, 10, 20, 30, 40, 49, HP]
            for k, (a, b) in enumerate(zip(cuts, cuts[1:])):
                q = nc.sync if k % 2 == 0 else nc.scalar
                q.dma_start(vt[0][:, :, :, a:b, :], v_d[0, :, :, :, a:b, :])
            for i in range(0, NI, 2):
                nc.sync.dma_start(usb[:, 1, i], u_d[:, 1, i])
                nc.scalar.dma_start(usb[:, 1, i + 1], u_d[:, 1, i + 1])
            for img in range(1, NPC):
                nc.sync.dma_start(vt[img][:, :, :, :31, :],
                                  v_d[img, :, :, :, :31, :])
                nc.scalar.dma_start(vt[img][:, :, :, 31:, :],
                                    v_d[img, :, :, :, 31:, :])

            zs = upool.tile([128, 512], BF16, name="zs")
            nc.gpsimd.memset(zs[:], 0.0)
            zp = pspool.tile([128, 512], F32, tag="pt", name="zp")
            for _ in range(12):
                nc.tensor.matmul(zp[:], zs[:, :128], zs[:], start=True,
                                 stop=True)

            ci = 0
            for img in range(NPC):
                chunks = [(0, 28), (28, 56)] if img < NPC - 1 else \
                         [(0, 28), (28, 42), (42, 56)]
                for (y0, y1) in chunks:
                    for oct_ in range(OCT):
                        rows = y1 - y0
                        pts = []
                        for i in range(NI):
                            pt = pspool.tile([128, rows, TQ], F32, tag="pt",
                                             name=f"pt{img}_{oct_}_{y0}_{i}")
                            pts.append(pt)
                            for dy in range(KH):
                                for ict in range(ICT):
                                    nc.tensor.matmul(
                                        pt[:],
                                        usb[:, oct_, i, dy, ict],
                                        vt[img][:, ict, i,
                                                y0 + dy:y0 + dy + rows, :],
                                        start=(dy == 0 and ict == 0),
                                        stop=(dy == KH - 1 and ict == ICT - 1),
                                    )
                        ot = opool.tile([128, rows, NA, TQ], F32, tag="ot",
                                        name=f"ot{img}_{oct_}_{y0}")

                        def tp(nm):
                            return tpool.tile([128, rows, TQ], F32, tag=nm,
                                              name=f"{nm}_{img}_{oct_}_{y0}")
                        c1, c3, c5 = tp("c1"), tp("c3"), tp("c5")
                        e, f, p, q = tp("e"), tp("f"), tp("p"), tp("q")
                        a1, qs, ps = tp("a1"), tp("qs"), tp("ps")
                        t1, t2, t3 = tp("t1"), tp("t2"), tp("t3")

                        nc.scalar.copy(c1[:], pts[1][:])
                        nc.scalar.copy(c3[:], pts[3][:])
                        nc.scalar.copy(c5[:], pts[5][:])
                        nc.vector.tensor_add(e[:], c1[:], pts[2][:])
                        nc.vector.tensor_sub(f[:], c1[:], pts[2][:])
                        nc.vector.tensor_add(p[:], c3[:], pts[4][:])
                        nc.vector.tensor_sub(q[:], c3[:], pts[4][:])
                        # Y0 = E + P + M0
                        nc.vector.tensor_add(a1[:], e[:], pts[0][:])
                        nc.gpsimd.tensor_add(ot[:, :, 0], a1[:], p[:])
                        # Y1 = 2*(F + 0.25*Q)
                        nc.scalar.mul(qs[:], q[:], 0.25)
                        nc.gpsimd.tensor_add(t1[:], f[:], qs[:])
                        nc.scalar.mul(ot[:, :, 1], t1[:], 2.0)
                        # Y2 = 4*(E + 0.0625*P)
                        nc.scalar.mul(ps[:], p[:], 0.0625)
                        nc.gpsimd.tensor_add(t2[:], e[:], ps[:])
                        nc.scalar.mul(ot[:, :, 2], t2[:], 4.0)
                        # Y3 = (Q*0.015625 + F)*8 + M5
                        nc.vector.scalar_tensor_tensor(t3[:], q[:], 0.015625,
                                                       f[:], MUL, ADD)
                        nc.vector.scalar_tensor_tensor(ot[:, :, 3], t3[:],
                                                       8.0, c5[:], MUL, ADD)
                        if y0 == 42:
                            # split the very last output DMA across queues
                            nc.sync.dma_start(
                                o_d[img, oct_ * 128:(oct_ + 1) * 128,
                                    y0:y0 + 7], ot[:, :7])
                            nc.scalar.dma_start(
                                o_d[img, oct_ * 128:(oct_ + 1) * 128,
                                    y0 + 7:y1], ot[:, 7:])
                        else:
                            out_eng = nc.sync if ci % 2 == 0 else nc.scalar
                            out_eng.dma_start(
                                o_d[img, oct_ * 128:(oct_ + 1) * 128, y0:y1],
                                ot[:])
                        ci += 1

    nc.compile()
    return nc


def _get_compiled():
    global _compiled
    if _compiled is None:
        _compiled = _build()
    return _compiled


def _prep_inputs(x, w):
    bf = ml_dtypes.bfloat16
    x = np.asarray(x, dtype=np.float32)
    w = np.asarray(w, dtype=np.float32)

    weff = w / (KH * KW)                                  # [oc, ic, dy, kx]
    U = np.stack([sum(G[i, k] * weff[..., k] for k in range(3))
                  for i in range(NI)], axis=2).astype(bf)  # [oc, ic, 6, 3]
    u = np.ascontiguousarray(
        U.reshape(OCT, 128, ICT, 128, NI, KH).transpose(3, 0, 4, 5, 2, 1))

    xp = np.zeros((N, IC, HP, W + 2), np.float32)
    xp[:, :, 1:H + 1, 1:W + 1] = x
    djs = [xp[..., j:j + 4 * (TQ - 1) + 1:4] for j in range(6)]
    V = np.stack([sum(BT[i, j] * djs[j] for j in range(6) if BT[i, j] != 0)
                  for i in range(NI)], axis=2).astype(bf)  # [n, ic, 6, 58, 14]
    v = np.ascontiguousarray(
        V.reshape(N, ICT, 128, NI, HP, TQ).transpose(0, 2, 1, 3, 4, 5))

    return [
        {"v": v[c * NPC:(c + 1) * NPC], "u": u}
        for c in range(N_CORES)
    ]


def kernel(x, w, _trace=False, _trace_kwargs=None):
    nc = _get_compiled()
    in_maps = _prep_inputs(x, w)
    res = run_bass_kernel_spmd(nc, in_maps, list(range(N_CORES)),
                               trace=_trace, **(_trace_kwargs or {}))
    o2 = np.concatenate([res.results[c]["out"] for c in range(N_CORES)],
                        axis=0)                    # [N, OC, H, 4(a), 14(t)]
    out = np.ascontiguousarray(
        o2.transpose(0, 1, 2, 4, 3)).reshape(N, OC, H, W)
    if _trace:
        return out, res
    return out


# revision 10
# speedup vs baseline: 1.1319x; 1.1319x over previous
"""Trainium2 Bass kernel for 3x3 same-padded conv (NCHW) scaled by 1/9.

v4: 1D Winograd F(4,3) along W, bf16, host-side input transform.
  - F(4,3) with Toom-Cook points {0, 2, -2, 1/2, -1/2} (chosen by CPU sweep:
    rel err 8.3e-3 vs 1.4e-2 for the classic {0,1,-1,2,-2}; gate is 2e-2).
    6 products per 4 outputs -> 2x less PE work than direct conv.
      f(x) = x^5 - 4.25 x^3 + x
      BT = [[1,0,-4.25,0,1,0], [0,-.5,-.25,2,1,0], [0,.5,-.25,-2,1,0],
            [0,-2,-4,.5,1,0],  [0,2,-4,-.5,1,0],  [0,1,0,-4.25,0,1]]
      G  = [[1,0,0], [1,2,4]/30, [1,-2,4]/30,
            [1,.5,.25]/-1.875, [1,-.5,.25]/-1.875, [0,0,1]]
      AT = [[1,1,1,1,1,0], [0,2,-2,.5,-.5,0], [0,4,4,.25,.25,0],
            [0,8,-8,.125,-.125,1]]
  - V = BT d computed ON HOST (fp32) -> bf16 [img, ic_p, ict, i, 58, 14].
    U = G w (1/9 folded) likewise.
  - Per (img, oct, 28-row chunk): 6 PSUM tiles M_i (FD=392) accumulate
    3 dy x 2 ict bf16 matmuls each (FWL keeps weight loads pipelined).
  - Output transform with E=M1+M2, F=M1-M2, P=M3+M4, Q=M3-M4:
      Y0 = M0+E+P; Y1 = 2F+.5Q = (F*4+Q)*.5; Y2 = 4E+.25P = (E*16+P)*.25;
      Y3 = 8F+.125Q+M5 = (Q*64+... -> t3=(Q*0.015625 + F); Y3 = t3*8 + M5
    ScalarE stages M1,M3 out of PSUM; DVE handles every op that reads PSUM
    (tensor_tensor allows only one PSUM input; GpSimd cannot touch PSUM);
    GpSimd does the SBUF-only coefficient combines.
"""

import numpy as np
import ml_dtypes

import concourse.bacc as bacc
import concourse.mybir as mybir
import concourse.tile as tile
from concourse.bass_utils import run_bass_kernel_spmd

N_CORES = 8
N, IC, H, W = 32, 256, 56, 56
OC, KH, KW = 256, 3, 3
NPC = N // N_CORES
ICT = IC // 128
OCT = OC // 128
HP = H + 2
TQ = 14                     # F(4,3) tiles per row (4 outputs each)
NI = 6
CHUNKS = [(0, 28), (28, 56)]

BF16 = mybir.dt.bfloat16
F32 = mybir.dt.float32
MUL = mybir.AluOpType.mult
ADD = mybir.AluOpType.add

BT = np.array([
    [1, 0, -4.25, 0, 1, 0],
    [0, -0.5, -0.25, 2, 1, 0],
    [0, 0.5, -0.25, -2, 1, 0],
    [0, -2, -4, 0.5, 1, 0],
    [0, 2, -4, -0.5, 1, 0],
    [0, 1, 0, -4.25, 0, 1],
], np.float32)
G = np.array([
    [1, 0, 0],
    [1 / 30, 2 / 30, 4 / 30],
    [1 / 30, -2 / 30, 4 / 30],
    [-8 / 15, -4 / 15, -2 / 15],
    [-8 / 15, 4 / 15, -2 / 15],
    [0, 0, 1],
], np.float32)

_compiled = None


def _build():
    nc = bacc.Bacc("TRN2", target_bir_lowering=False, debug=False,
                   num_devices=N_CORES)

    v_d = nc.dram_tensor("v", [NPC, 128, ICT, NI, HP, TQ], BF16,
                         kind="ExternalInput")
    u_d = nc.dram_tensor("u", [128, OCT, NI, KH, ICT, 128], BF16,
                         kind="ExternalInput")
    o_d = nc.dram_tensor("out", [NPC, OC, H, W], F32, kind="ExternalOutput")

    with tile.TileContext(nc) as tc:
        with (
            tc.tile_pool(name="vp", bufs=1) as vpool,
            tc.tile_pool(name="up", bufs=1) as upool,
            tc.tile_pool(name="tp", bufs=4) as tpool,
            tc.tile_pool(name="op", bufs=4) as opool,
            tc.tile_pool(name="ps", bufs=8, space="PSUM") as pspool,
        ):
            usb = upool.tile([128, OCT, NI, KH, ICT, 128], BF16, name="usb")
            for i in range(NI):
                nc.sync.dma_start(usb[:, 0, i], u_d[:, 0, i])

            vt = []
            for img in range(NPC):
                vt.append(vpool.tile([128, ICT, NI, HP, TQ], BF16,
                                     tag=f"v{img}", name=f"v{img}"))
            # img0 in 4 row-pieces alternating queues so chunk-0 matmuls
            # (need V rows 0..29) can start after the first two pieces.
            cuts = [0, 15, 30, 44, HP]
            for k, (a, b) in enumerate(zip(cuts, cuts[1:])):
                q = nc.scalar if k % 2 == 0 else nc.sync
                q.dma_start(vt[0][:, :, :, a:b, :], v_d[0, :, :, :, a:b, :])
            for i in range(NI):
                nc.sync.dma_start(usb[:, 1, i], u_d[:, 1, i])
            nc.sync.dma_start(vt[1][:], v_d[1])
            nc.scalar.dma_start(vt[2][:], v_d[2])
            nc.sync.dma_start(vt[3][:], v_d[3])

            zs = upool.tile([128, 512], BF16, name="zs")
            nc.gpsimd.memset(zs[:], 0.0)
            zp = pspool.tile([128, 512], F32, tag="pt", name="zp")
            for _ in range(18):
                nc.tensor.matmul(zp[:], zs[:, :128], zs[:], start=True,
                                 stop=True)

            ci = 0
            for img in range(NPC):
                for oct_ in range(OCT):
                    for (y0, y1) in CHUNKS:
                        rows = y1 - y0
                        pts = []
                        for i in range(NI):
                            pt = pspool.tile([128, rows, TQ], F32, tag="pt",
                                             name=f"pt{img}_{oct_}_{y0}_{i}")
                            pts.append(pt)
                            for dy in range(KH):
                                for ict in range(ICT):
                                    nc.tensor.matmul(
                                        pt[:],
                                        usb[:, oct_, i, dy, ict],
                                        vt[img][:, ict, i,
                                                y0 + dy:y0 + dy + rows, :],
                                        start=(dy == 0 and ict == 0),
                                        stop=(dy == KH - 1 and ict == ICT - 1),
                                    )
                        ot = opool.tile([128, rows, W], F32, tag="ot",
                                        name=f"ot{img}_{oct_}_{y0}")

                        def tp(nm):
                            return tpool.tile([128, rows, TQ], F32, tag=nm,
                                              name=f"{nm}_{img}_{oct_}_{y0}")
                        c1, c3, c5 = tp("c1"), tp("c3"), tp("c5")
                        e, f, p, q = tp("e"), tp("f"), tp("p"), tp("q")
                        a1, qs, ps = tp("a1"), tp("qs"), tp("ps")
                        t1, t2, t3 = tp("t1"), tp("t2"), tp("t3")

                        # Engine constraints: tensor_tensor max 1 PSUM input;
                        # GpSimd cannot access PSUM; TensorScalarPtr (stt,
                        # tensor_scalar) not supported on GpSimd. Split:
                        # ScalarE copies/scales, DVE does PSUM ops + stt,
                        # GpSimd plain SBUF adds.
                        nc.scalar.copy(c1[:], pts[1][:])
                        nc.scalar.copy(c3[:], pts[3][:])
                        nc.scalar.copy(c5[:], pts[5][:])
                        nc.vector.tensor_add(e[:], c1[:], pts[2][:])
                        nc.vector.tensor_sub(f[:], c1[:], pts[2][:])
                        nc.vector.tensor_add(p[:], c3[:], pts[4][:])
                        nc.vector.tensor_sub(q[:], c3[:], pts[4][:])
                        # Y0 = E + P + M0
                        nc.vector.tensor_add(a1[:], e[:], pts[0][:])
                        nc.gpsimd.tensor_add(ot[:, :, 0::4], a1[:], p[:])
                        # Y1 = 2*(F + 0.25*Q)
                        nc.scalar.mul(qs[:], q[:], 0.25)
                        nc.gpsimd.tensor_add(t1[:], f[:], qs[:])
                        nc.scalar.mul(ot[:, :, 1::4], t1[:], 2.0)
                        # Y2 = 4*(E + 0.0625*P)
                        nc.scalar.mul(ps[:], p[:], 0.0625)
                        nc.gpsimd.tensor_add(t2[:], e[:], ps[:])
                        nc.scalar.mul(ot[:, :, 2::4], t2[:], 4.0)
                        # Y3 = (Q*0.015625 + F)*8 + M5
                        nc.vector.scalar_tensor_tensor(t3[:], q[:], 0.015625,
                                                       f[:], MUL, ADD)
                        nc.vector.scalar_tensor_tensor(ot[:, :, 3::4], t3[:],
                                                       8.0, c5[:], MUL, ADD)
                        out_eng = nc.sync if ci % 2 == 0 else nc.scalar
                        out_eng.dma_start(
                            o_d[img, oct_ * 128:(oct_ + 1) * 128, y0:y1, :],
                            ot[:])
                        ci += 1

    nc.compile()
    return nc


def _get_compiled():
    global _compiled
    if _compiled is None:
        _compiled = _build()
    return _compiled


def _prep_inputs(x, w):
    bf = ml_dtypes.bfloat16
    x = np.asarray(x, dtype=np.float32)
    w = np.asarray(w, dtype=np.float32)

    weff = w / (KH * KW)                                  # [oc, ic, dy, kx]
    U = np.stack([sum(G[i, k] * weff[..., k] for k in range(3))
                  for i in range(NI)], axis=2).astype(bf)  # [oc, ic, 6, 3]
    u = np.ascontiguousarray(
        U.reshape(OCT, 128, ICT, 128, NI, KH).transpose(3, 0, 4, 5, 2, 1))

    xp = np.zeros((N, IC, HP, W + 2), np.float32)
    xp[:, :, 1:H + 1, 1:W + 1] = x
    djs = [xp[..., j:j + 4 * (TQ - 1) + 1:4] for j in range(6)]
    V = np.stack([sum(BT[i, j] * djs[j] for j in range(6) if BT[i, j] != 0)
                  for i in range(NI)], axis=2).astype(bf)  # [n, ic, 6, 58, 14]
    v = np.ascontiguousarray(
        V.reshape(N, ICT, 128, NI, HP, TQ).transpose(0, 2, 1, 3, 4, 5))

    return [
        {"v": v[c * NPC:(c + 1) * NPC], "u": u}
        for c in range(N_CORES)
    ]


def kernel(x, w, _trace=False, _trace_kwargs=None):
    nc = _get_compiled()
    in_maps = _prep_inputs(x, w)
    res = run_bass_kernel_spmd(nc, in_maps, list(range(N_CORES)),
                               trace=_trace, **(_trace_kwargs or {}))
    out = np.concatenate([res.results[c]["out"] for c in range(N_CORES)],
                         axis=0)
    if _trace:
        return out, res
    return out
